# revision 38
# baseline (speedup 1.0000x reference)
"""Multi-head self-attention (B=2, S=2048, E=1024, H=16, D=64, causal) on 8 trn2 cores.

Sharding: tensor-parallel over (batch, head-group). Core c handles batch c//4 and
heads [4*(c%4), 4*(c%4)+4). Each core computes QKV projection for its 4 heads,
causal flash-attention, and a partial output projection (its heads' rows of
w_out). Host sums the 4 partials per batch and adds b_out.

Device math (per core, matmuls in bf16):
  qT/kT [j, s] = (wqk_ext).T @ xT          (j on partitions -> scores need no transpose)
  v [s, j]     = xT.T @ wv                 (s-block on partitions)
  S^T tile [sk, sq] = kT.T-slice @ qT-slice  (two heads row-tiled on the PE, run
  concurrently via tile_position=(0,0)/(64,0))
  P^T = exp(S^T / 8) with causal triangle mask; no max-subtraction needed
  (scores ~ N(0,1), exp bounded ~e^6, fp32-safe)
  PV: [O^T; L] = v_ext.T @ P^T accumulated over sk chunks; L = softmax denominator
  (v_ext has interleaved ones-columns, filled by memset)
  O^T normalized by 1/L, projected: out_partial = OT.T @ wout_rows (bf16 to host)

Engine split: PE = all matmuls (~105us), ACT = exp only, DVE = all PSUM
evictions + reciprocal + normalization + causal mask muls, GPSIMD = memset.
"""
import sys

sys.path.insert(0, "/opt/trn_rl_repo")

import ml_dtypes
import numpy as np

import concourse.bacc as bacc
import concourse.mybir as mybir
import concourse.tile as tile



B, S, E = 2, 2048, 1024
H, D = 16, 64
HPC = 4          # heads per core
NCORES = 8
SC = 512         # sq chunk width (scores free dim)
KC = 128         # sk chunk width
NQC = S // SC    # 4 q-chunks
NSB = S // 128   # 16 s-blocks

f32 = mybir.dt.float32
bf16 = mybir.dt.bfloat16

_NC = None


def _build_nc():
    nc = bacc.Bacc(None, target_bir_lowering=False)

    xT = nc.dram_tensor("xT", [E, S], bf16, kind="ExternalInput")
    wqk = nc.dram_tensor("wqk", [128, 4, 8, 128], bf16, kind="ExternalInput")
    wv = nc.dram_tensor("wv", [128, 8, 256], bf16, kind="ExternalInput")
    wout = nc.dram_tensor("wout", [128, 2, E], bf16, kind="ExternalInput")
    mask2 = nc.dram_tensor("mask2", [128, 256], bf16, kind="ExternalInput")
    out_p = nc.dram_tensor("out_p", [S, E], bf16, kind="ExternalOutput")

    with tile.TileContext(nc) as tc:
        with (
            tc.tile_pool(name="big", bufs=1) as big,
            tc.tile_pool(name="ptp", bufs=4) as ptp,
            tc.tile_pool(name="lvp", bufs=2) as lvp,
            tc.tile_pool(name="osb", bufs=3) as osbp,
            tc.tile_pool(name="psP", bufs=2, space="PSUM") as psP,
            tc.tile_pool(name="psST", bufs=2, space="PSUM") as psST,
            tc.tile_pool(name="psPV", bufs=1, space="PSUM") as psPV,
        ):
            xT_sb = big.tile([128, 8, S], bf16)
            wqk_sb = big.tile([128, 4, 8, 128], bf16)
            wu_sb = big.tile([128, 512], bf16)
            wv_sb = big.tile([128, 8, 256], bf16)
            qkT_sb = big.tile([128, 4, S], bf16)
            v_sb = big.tile([128, NSB, 512], bf16)
            OT_sb = big.tile([128, 2, S], bf16)
            wout_sb = big.tile([128, 2, E], bf16)
            mask2_sb = big.tile([128, 256], bf16)

            # ---- PE warmup: ~3.5us of dummy matmuls during the input-DMA
            # phase releases the HAM clock gate (1.2 -> 2.4 GHz) before real
            # work arrives ----
            nc.vector.memset(wu_sb, 1.0)
            wu_ps = psST.tile([128, 1024], f32, tag="ST", name="wu_ps")
            NWU, NWU2 = 13, 20
            for i in range(NWU):
                nc.tensor.matmul(wu_ps[:, 0:512], wu_sb[:, 0:128], wu_sb,
                                 start=(i == 0), stop=False)
            # taper with short matmuls: keeps the PE active (HAM warm) until
            # the first input DMAs land, at minimal queue-delay cost
            for i in range(NWU2):
                nc.tensor.matmul(wu_ps[:, 0:128], wu_sb[:, 0:128], wu_sb[:, 0:128],
                                 start=False, stop=(i == NWU2 - 1))

            # ---- input DMAs (priority order: pair0 deps first) ----
            # wqk is host-packed jb-major: wqk[:, jb] is 2KB contiguous per
            # partition, so the jb=0/2 slices (first q,k matmuls) land first.
            # first deps dispatched on BOTH DMA-capable sequencers (sync +
            # scalar) so their ~0.5us dispatch costs overlap
            nc.sync.dma_start(out=wqk_sb[:, 0], in_=wqk[:, 0])
            # xT sc0 chunk (kc-split quarters): needed by qk_sc(*, 0), v_block(0..3)
            for kq in range(4):
                eng = nc.scalar if kq % 2 == 0 else nc.sync
                eng.dma_start(
                    out=xT_sb[:, 2 * kq:2 * kq + 2, 0:SC],
                    in_=xT[256 * kq:256 * (kq + 1), 0:SC].rearrange(
                        "(k p) f -> p k f", p=128))
                if kq == 0:
                    nc.sync.dma_start(out=wqk_sb[:, 2], in_=wqk[:, 2])
            nc.scalar.dma_start(out=mask2_sb, in_=mask2[:, :])
            nc.sync.dma_start(out=wv_sb, in_=wv[:, :, :])
            # rest of xT staged per sc-chunk so qk_sc(*, sc)/v_block filler
            # unblocks progressively during the first attention units
            nc.sync.dma_start(
                out=xT_sb[:, :, SC:2 * SC],
                in_=xT[:, SC:2 * SC].rearrange("(k p) f -> p k f", p=128))
            nc.sync.dma_start(out=wqk_sb[:, 1], in_=wqk[:, 1])
            nc.sync.dma_start(out=wqk_sb[:, 3], in_=wqk[:, 3])
            nc.sync.dma_start(
                out=xT_sb[:, :, 2 * SC:3 * SC],
                in_=xT[:, 2 * SC:3 * SC].rearrange("(k p) f -> p k f", p=128))
            nc.sync.dma_start(
                out=xT_sb[:, :, 3 * SC:S],
                in_=xT[:, 3 * SC:S].rearrange("(k p) f -> p k f", p=128))
            nc.sync.dma_start(out=wout_sb, in_=wout[:, :, :])
            # ones columns of v_ext: [64:192] and [320:448] within each 512 block
            nc.gpsimd.memset(v_sb[:, :, 64:192], 1.0)
            nc.gpsimd.memset(v_sb[:, :, 320:448], 1.0)

            # ---- QKV projection ----
            def qk_sc(jb, sc):
                # qkT_sb[:, jb, sc] = wqk[:, jb*128:+128].T @ xT[:, sc]
                ps = psP.tile([128, SC], f32, tag="P", name="ps_qk")
                for kc in range(8):
                    nc.tensor.matmul(
                        ps[:, :],
                        wqk_sb[:, jb, kc, :],
                        xT_sb[:, kc, sc * SC:(sc + 1) * SC],
                        start=(kc == 0), stop=(kc == 7))
                nc.vector.tensor_copy(qkT_sb[:, jb, sc * SC:(sc + 1) * SC], ps[:, :])

            def v_block(sb):
                # raw v [128, 256] = xT[:, sb*128:+128].T @ wv; heads h0..h3, 64 cols each.
                # v_ext per head pair: [v_e | ones | ones | v_o]; v cols land at
                # {0:64, 192:256} + 256*pp.
                ps = psP.tile([128, SC], f32, tag="P", name="ps_v")
                for kc in range(8):
                    nc.tensor.matmul(
                        ps[:, 0:256],
                        xT_sb[:, kc, sb * 128:(sb + 1) * 128],
                        wv_sb[:, kc, :],
                        start=(kc == 0), stop=(kc == 7))
                ps4 = ps[:, 0:256].rearrange("p (a b c) -> p a b c", a=2, b=2)  # [2pair, 2side, 64]
                vs4 = v_sb[:, sb, :].rearrange("p (a c) -> p a c", a=2)         # [2pair, 256]
                # even heads (h0,h2): ps [pair, 0, :] -> v_sb cols [0:64] of each pair
                nc.vector.tensor_copy(vs4[:, :, 0:64], ps4[:, :, 0, :])
                # odd heads (h1,h3): ps [pair, 1, :] -> v_sb cols [192:256] of each pair
                nc.vector.tensor_copy(vs4[:, :, 192:256], ps4[:, :, 1, :])

            # ---- attention for one (head pair, q-chunk) ----
            def attention_qc(pair, qc, tail=False):
                qblk, kblk = pair, 2 + pair
                nkc = 4 * qc + 4
                pv = psPV.tile([128, 1024], f32, tag="PV", name="pv")

                def scores_exp(kc):
                    # diagonal tiles (r >= 0): columns < 128*r are causally
                    # invalid -- skip them in the matmul, exp, and PV (ragged).
                    r = kc - 4 * qc
                    off = KC * r if r > 0 else 0
                    st = psST.tile([128, 1024], f32, tag="ST", name="st")
                    nc.tensor.matmul(
                        st[:, off:SC],
                        qkT_sb[0:64, kblk, kc * KC:(kc + 1) * KC],
                        qkT_sb[0:64, qblk, qc * SC + off:(qc + 1) * SC],
                        start=True, stop=True, tile_position=(0, 0))
                    nc.tensor.matmul(
                        st[:, SC + off:1024],
                        qkT_sb[64:128, kblk, kc * KC:(kc + 1) * KC],
                        qkT_sb[64:128, qblk, qc * SC + off:(qc + 1) * SC],
                        start=True, stop=True, tile_position=(64, 0))
                    pt = ptp.tile([128, 1024], bf16, name="pt")
                    if r < 0:
                        nc.scalar.activation(
                            out=pt[:, :], in_=st[:, :],
                            func=mybir.ActivationFunctionType.Exp, scale=0.125)
                    else:
                        # one strided exp covering both heads' valid ranges
                        st2 = st.rearrange("p (h f) -> p h f", h=2)
                        pt2 = pt.rearrange("p (h f) -> p h f", h=2)
                        nc.scalar.activation(
                            out=pt2[:, :, off:SC], in_=st2[:, :, off:SC],
                            func=mybir.ActivationFunctionType.Exp, scale=0.125)
                        # causal triangle mask on the diagonal 128-block of each head
                        tri = pt2[:, :, off:off + KC]
                        m2 = mask2_sb.rearrange("p (h f) -> p h f", h=2)
                        nc.vector.tensor_mul(tri, tri, m2)
                    return pt

                def pv_step(kc, pt):
                    r = kc - 4 * qc
                    off = KC * r if r > 0 else 0
                    for h2 in range(2):
                        hh = 2 * pair + h2
                        nc.tensor.matmul(
                            pv[:, SC * h2 + off:SC * h2 + SC],
                            v_sb[:, kc, 128 * hh:128 * hh + 128],
                            pt[:, SC * h2 + off:SC * h2 + SC],
                            start=(kc == 0), stop=(kc == nkc - 1))

                # process kc in pairs: two score pairs (64-row tiling mode),
                # then two pv pairs (full-array mode) -- halves the PE
                # tiling-mode switches, each of which drains the array
                pts = {}
                for kc2 in range(0, nkc, 2):
                    pts[kc2] = scores_exp(kc2)
                    pts[kc2 + 1] = scores_exp(kc2 + 1)
                    if kc2 >= 2:
                        pv_step(kc2 - 2, pts.pop(kc2 - 2))
                        pv_step(kc2 - 1, pts.pop(kc2 - 1))
                pv_step(nkc - 2, pts.pop(nkc - 2))
                pv_step(nkc - 1, pts.pop(nkc - 1))

                # normalization: even head [v|ones] -> O rows 0:64 / L rows 64:128
                # of bank0; odd head [ones|v] -> L rows 0:64 / O rows 64:128 of
                # bank1. reciprocal_approx_fast is broken at base_partition != 0,
                # so read full 128 partitions (unused rows produce garbage that
                # is never consumed).
                qs = qc * SC
                rec = lvp.tile([128, 1024], f32, tag="rec", name="rec")
                linv = lvp.tile([128, SC], f32, tag="linv", name="linv")
                # the tail unit normalizes in sq-halves so the final proj
                # chunks can start while the second half is still going
                if tail:
                    for lo, hi in ((0, 256), (256, SC)):
                        nc.vector.reciprocal_approx_fast(
                            out=rec[:, lo:hi], in_=pv[:, lo:hi])
                        nc.sync.dma_start(
                            out=linv[0:64, lo:hi], in_=rec[64:128, lo:hi])
                        nc.vector.reciprocal_approx_fast(
                            out=rec[:, SC + lo:SC + hi], in_=pv[:, SC + lo:SC + hi])
                        nc.sync.dma_start(
                            out=linv[64:128, lo:hi], in_=rec[0:64, SC + lo:SC + hi])
                        nc.vector.tensor_mul(
                            OT_sb[0:64, pair, qs + lo:qs + hi],
                            pv[0:64, lo:hi], linv[0:64, lo:hi])
                        nc.vector.tensor_mul(
                            OT_sb[64:128, pair, qs + lo:qs + hi],
                            pv[64:128, SC + lo:SC + hi], linv[64:128, lo:hi])
                else:
                    nc.vector.reciprocal_approx_fast(out=rec, in_=pv[:, :])
                    nc.sync.dma_start(out=linv[0:64, :], in_=rec[64:128, 0:SC])
                    nc.sync.dma_start(out=linv[64:128, :], in_=rec[0:64, SC:1024])
                    nc.vector.tensor_mul(
                        OT_sb[0:64, pair, qs:qs + SC], pv[0:64, 0:SC], linv[0:64, :])
                    nc.vector.tensor_mul(
                        OT_sb[64:128, pair, qs:qs + SC],
                        pv[64:128, SC:1024], linv[64:128, :])

            # ---- partial output projection (one 128-row s-block; two
            # [128,512] PSUM tiles so the pool stays one-bank granular) ----
            def proj_chunk(sc, tail=False):
                osb = osbp.tile([128, E], bf16, name="osb")
                if tail:
                    # attention is done by now: the psST banks are free, so
                    # use a full 2-bank tile and a single eviction
                    po2 = psST.tile([128, 1024], f32, tag="ST", name="po2")
                    for nh in range(2):
                        for p in range(2):
                            nc.tensor.matmul(
                                po2[:, SC * nh:SC * nh + SC],
                                OT_sb[:, p, sc * 128:(sc + 1) * 128],
                                wout_sb[:, p, SC * nh:SC * nh + SC],
                                start=(p == 0), stop=(p == 1))
                    nc.vector.tensor_copy(osb, po2)
                else:
                    for nh in range(2):
                        po = psP.tile([128, SC], f32, tag="P", name="po")
                        for p in range(2):
                            nc.tensor.matmul(
                                po[:, :],
                                OT_sb[:, p, sc * 128:(sc + 1) * 128],
                                wout_sb[:, p, SC * nh:SC * nh + SC],
                                start=(p == 0), stop=(p == 1))
                        nc.vector.tensor_copy(osb[:, SC * nh:SC * nh + SC], po[:, :])
                nc.sync.dma_start(out=out_p[sc * 128:(sc + 1) * 128, :], in_=osb)

            # ---- emission schedule: start attention ASAP; keep PE fed with
            # projection/v filler while ACT (exp) gates attention; spread proj
            # chunks through the run instead of a tail ----
            # ---- emission order = data program order (tile tracks deps by
            # emission sequence), so filler must be emitted before its
            # attention consumers. Scheduling PRIORITY is separate: each
            # attention unit is wrapped in high_priority(offset) so its
            # matmuls/exps beat ready filler ops in the per-engine heaps,
            # making qkv/proj pure gap-filler. ----
            ATTN_PRIO = 100000

            def attn(pair, qc, tail=False):
                with tc.high_priority(offset=ATTN_PRIO):
                    attention_qc(pair, qc, tail=tail)

            qk_sc(0, 0); qk_sc(2, 0)
            for sb in range(4):
                v_block(sb)
            qk_sc(0, 1); qk_sc(2, 1)
            attn(0, 0)
            for sb in range(4, 8):
                v_block(sb)
            attn(0, 1)
            qk_sc(1, 0); qk_sc(3, 0)
            for sb in range(8, 12):
                v_block(sb)
            attn(1, 0)
            qk_sc(0, 2); qk_sc(2, 2)
            attn(0, 2)
            for sb in range(12, 16):
                v_block(sb)
            for sc in range(4):
                proj_chunk(sc)
            qk_sc(1, 1); qk_sc(3, 1)
            attn(1, 1)
            qk_sc(0, 3); qk_sc(2, 3)
            attn(0, 3)
            qk_sc(1, 2); qk_sc(3, 2)
            for sc in range(4, 8):
                proj_chunk(sc)
            attn(1, 2)
            qk_sc(1, 3); qk_sc(3, 3)
            for sc in range(8, 12):
                proj_chunk(sc)
            attn(1, 3, tail=True)
            for sc in range(12, 16):
                proj_chunk(sc, tail=True)

    nc.finalize()
    return nc


def _get_nc():
    global _NC
    if _NC is None:
        _NC = _build_nc()
    return _NC


def _pack_rows(w):
    """[R, C] -> [128, R//128, C]: partition p holds rows p, 128+p, ..."""
    r, c = w.shape
    return np.ascontiguousarray(
        w.reshape(r // 128, 128, c).transpose(1, 0, 2)).astype(ml_dtypes.bfloat16)


def _prep_in_maps(x, w_qkv, b_qkv):
    x = np.asarray(x, dtype=np.float32)
    w_qkv = np.asarray(w_qkv, dtype=np.float32)

    xT_by_batch = [np.ascontiguousarray(x[b].T).astype(ml_dtypes.bfloat16) for b in range(B)]

    tri = np.triu(np.ones((128, 128), dtype=np.float32))  # valid where sq >= sk
    mask2 = np.concatenate([tri, tri], axis=1).astype(ml_dtypes.bfloat16)

    in_maps = []
    for c in range(NCORES):
        b, g = divmod(c, HPC)
        h0 = HPC * g  # first global head for this core
        cq = slice(h0 * D, (h0 + HPC) * D)
        ck = slice(H * D + h0 * D, H * D + (h0 + HPC) * D)

        wqk = np.empty((E, 512), dtype=np.float32)
        wqk[:, 0:256] = w_qkv[:, cq]
        wqk[:, 256:512] = w_qkv[:, ck]
        # pack jb-major: [p, jb, kc, c] so each jb slice is contiguous per row
        wqk_p = np.ascontiguousarray(
            wqk.reshape(8, 128, 4, 128).transpose(1, 2, 0, 3)
        ).astype(ml_dtypes.bfloat16)

        # b_qkv is zeros by the problem spec (fill: zeros); the device program
        # has no bias path.
        cv = slice(2 * H * D + h0 * D, 2 * H * D + (h0 + HPC) * D)

        in_maps.append({
            "xT": xT_by_batch[b],
            "wqk": wqk_p,
            "wv": _pack_rows(w_qkv[:, cv]),
            "wout": None,  # filled by caller (needs w_out)
            "mask2": mask2,
        })
    return in_maps


def run(x, w_qkv, b_qkv, w_out, b_out, trace=False, **spmd_kwargs):
    from concourse.bass_utils import run_bass_kernel_spmd

    w_out = np.asarray(w_out, dtype=np.float32)
    b_out = np.asarray(b_out, dtype=np.float32)
    in_maps = _prep_in_maps(x, w_qkv, b_qkv)
    for c in range(NCORES):
        h0 = HPC * (c % HPC)
        in_maps[c]["wout"] = _pack_rows(w_out[h0 * D:(h0 + HPC) * D, :])

    nc = _get_nc()
    res = run_bass_kernel_spmd(nc, in_maps, core_ids=list(range(NCORES)),
                               trace=trace, **spmd_kwargs)
    out = np.empty((B, S, E), dtype=np.float32)
    for b in range(B):
        acc = res.results[HPC * b]["out_p"].astype(np.float32)
        for i in range(1, HPC):
            acc = acc + res.results[HPC * b + i]["out_p"].astype(np.float32)
        out[b] = acc + b_out
    return out, res


def kernel(x, w_qkv, b_qkv, w_out, b_out):
    out, _ = run(x, w_qkv, b_qkv, w_out, b_out, trace=False)
    return out


# revision 39
# speedup vs baseline: 1.0367x; 1.0367x over previous
"""Multi-head self-attention (B=2, S=2048, E=1024, H=16, D=64, causal) on 8 trn2 cores.

Sharding: tensor-parallel over (batch, head-group). Core c handles batch c//4 and
heads [4*(c%4), 4*(c%4)+4). Each core computes QKV projection for its 4 heads,
causal flash-attention, and a partial output projection (its heads' rows of
w_out). Host sums the 4 partials per batch and adds b_out.

Device math (per core, matmuls in bf16):
  qT/kT [j, s] = (wqk_ext).T @ xT          (j on partitions -> scores need no transpose)
  v [s, j]     = xT.T @ wv                 (s-block on partitions)
  S^T tile [sk, sq] = kT.T-slice @ qT-slice  (two heads row-tiled on the PE, run
  concurrently via tile_position=(0,0)/(64,0))
  P^T = exp(S^T / 8) with causal triangle mask; no max-subtraction needed
  (scores ~ N(0,1), exp bounded ~e^6, fp32-safe)
  PV: [O^T; L] = v_ext.T @ P^T accumulated over sk chunks; L = softmax denominator
  (v_ext has interleaved ones-columns, filled by memset)
  O^T normalized by 1/L, projected: out_partial = OT.T @ wout_rows (bf16 to host)

Engine split: PE = all matmuls (~105us), ACT = exp only, DVE = all PSUM
evictions + reciprocal + normalization + causal mask muls, GPSIMD = memset.
"""
import sys

sys.path.insert(0, "/opt/trn_rl_repo")

import ml_dtypes
import numpy as np

import concourse.bacc as bacc
import concourse.mybir as mybir
import concourse.tile as tile



B, S, E = 2, 2048, 1024
H, D = 16, 64
HPC = 4          # heads per core
NCORES = 8
SC = 512         # sq chunk width (scores free dim)
KC = 128         # sk chunk width
NQC = S // SC    # 4 q-chunks
NSB = S // 128   # 16 s-blocks

f32 = mybir.dt.float32
bf16 = mybir.dt.bfloat16

_NC = None


def _build_nc():
    nc = bacc.Bacc(None, target_bir_lowering=False)

    xT = nc.dram_tensor("xT", [E, S], bf16, kind="ExternalInput")
    wqk = nc.dram_tensor("wqk", [128, 4, 8, 128], bf16, kind="ExternalInput")
    wv = nc.dram_tensor("wv", [128, 8, 256], bf16, kind="ExternalInput")
    wout = nc.dram_tensor("wout", [128, 2, E], bf16, kind="ExternalInput")
    mask2 = nc.dram_tensor("mask2", [128, 256], bf16, kind="ExternalInput")
    out_p = nc.dram_tensor("out_p", [S, E], bf16, kind="ExternalOutput")

    with tile.TileContext(nc) as tc:
        with (
            tc.tile_pool(name="big", bufs=1) as big,
            tc.tile_pool(name="ptp", bufs=4) as ptp,
            tc.tile_pool(name="lvp", bufs=2) as lvp,
            tc.tile_pool(name="osb", bufs=3) as osbp,
            tc.tile_pool(name="psP", bufs=2, space="PSUM") as psP,
            tc.tile_pool(name="psST", bufs=2, space="PSUM") as psST,
            tc.tile_pool(name="psPV", bufs=1, space="PSUM") as psPV,
        ):
            xT_sb = big.tile([128, 8, S], bf16)
            wqk_sb = big.tile([128, 4, 8, 128], bf16)
            wu_sb = big.tile([128, 512], bf16)
            wv_sb = big.tile([128, 8, 256], bf16)
            qkT_sb = big.tile([128, 4, S], bf16)
            v_sb = big.tile([128, NSB, 512], bf16)
            OT_sb = big.tile([128, 2, S], bf16)
            wout_sb = big.tile([128, 2, E], bf16)
            mask2_sb = big.tile([128, 256], bf16)

            # ---- PE warmup: ~3.5us of dummy matmuls during the input-DMA
            # phase releases the HAM clock gate (1.2 -> 2.4 GHz) before real
            # work arrives ----
            nc.vector.memset(wu_sb, 1.0)
            wu_ps = psST.tile([128, 1024], f32, tag="ST", name="wu_ps")
            NWU, NWU2 = 13, 20
            for i in range(NWU):
                nc.tensor.matmul(wu_ps[:, 0:512], wu_sb[:, 0:128], wu_sb,
                                 start=(i == 0), stop=False)
            # taper with short matmuls: keeps the PE active (HAM warm) until
            # the first input DMAs land, at minimal queue-delay cost
            for i in range(NWU2):
                nc.tensor.matmul(wu_ps[:, 0:128], wu_sb[:, 0:128], wu_sb[:, 0:128],
                                 start=False, stop=(i == NWU2 - 1))

            # ---- input DMAs (priority order: pair0 deps first) ----
            # wqk is host-packed jb-major: wqk[:, jb] is 2KB contiguous per
            # partition, so the jb=0/2 slices (first q,k matmuls) land first.
            # first deps dispatched on BOTH DMA-capable sequencers (sync +
            # scalar) so their ~0.5us dispatch costs overlap
            nc.sync.dma_start(out=wqk_sb[:, 0], in_=wqk[:, 0])
            # xT sc0 chunk (kc-split quarters): needed by qk_sc(*, 0), v_block(0..3)
            for kq in range(4):
                eng = nc.scalar if kq % 2 == 0 else nc.sync
                eng.dma_start(
                    out=xT_sb[:, 2 * kq:2 * kq + 2, 0:SC],
                    in_=xT[256 * kq:256 * (kq + 1), 0:SC].rearrange(
                        "(k p) f -> p k f", p=128))
                if kq == 0:
                    nc.sync.dma_start(out=wqk_sb[:, 2], in_=wqk[:, 2])
            nc.scalar.dma_start(out=mask2_sb, in_=mask2[:, :])
            nc.sync.dma_start(out=wv_sb, in_=wv[:, :, :])
            # rest of xT staged per sc-chunk so qk_sc(*, sc)/v_block filler
            # unblocks progressively during the first attention units
            nc.sync.dma_start(
                out=xT_sb[:, :, SC:2 * SC],
                in_=xT[:, SC:2 * SC].rearrange("(k p) f -> p k f", p=128))
            nc.sync.dma_start(out=wqk_sb[:, 1], in_=wqk[:, 1])
            nc.sync.dma_start(out=wqk_sb[:, 3], in_=wqk[:, 3])
            nc.sync.dma_start(
                out=xT_sb[:, :, 2 * SC:3 * SC],
                in_=xT[:, 2 * SC:3 * SC].rearrange("(k p) f -> p k f", p=128))
            nc.sync.dma_start(
                out=xT_sb[:, :, 3 * SC:S],
                in_=xT[:, 3 * SC:S].rearrange("(k p) f -> p k f", p=128))
            nc.sync.dma_start(out=wout_sb, in_=wout[:, :, :])
            # ones columns of v_ext: [64:192] and [320:448] within each 512 block
            nc.gpsimd.memset(v_sb[:, :, 64:192], 1.0)
            nc.gpsimd.memset(v_sb[:, :, 320:448], 1.0)

            # ---- QKV projection ----
            def qk_sc(jb, sc):
                # qkT_sb[:, jb, sc] = wqk[:, jb*128:+128].T @ xT[:, sc]
                ps = psP.tile([128, SC], f32, tag="P", name="ps_qk")
                for kc in range(8):
                    nc.tensor.matmul(
                        ps[:, :],
                        wqk_sb[:, jb, kc, :],
                        xT_sb[:, kc, sc * SC:(sc + 1) * SC],
                        start=(kc == 0), stop=(kc == 7))
                nc.any.tensor_copy(qkT_sb[:, jb, sc * SC:(sc + 1) * SC], ps[:, :])

            def v_block(sb):
                # raw v [128, 256] = xT[:, sb*128:+128].T @ wv; heads h0..h3, 64 cols each.
                # v_ext per head pair: [v_e | ones | ones | v_o]; v cols land at
                # {0:64, 192:256} + 256*pp.
                ps = psP.tile([128, SC], f32, tag="P", name="ps_v")
                for kc in range(8):
                    nc.tensor.matmul(
                        ps[:, 0:256],
                        xT_sb[:, kc, sb * 128:(sb + 1) * 128],
                        wv_sb[:, kc, :],
                        start=(kc == 0), stop=(kc == 7))
                ps4 = ps[:, 0:256].rearrange("p (a b c) -> p a b c", a=2, b=2)  # [2pair, 2side, 64]
                vs4 = v_sb[:, sb, :].rearrange("p (a c) -> p a c", a=2)         # [2pair, 256]
                # even heads (h0,h2): ps [pair, 0, :] -> v_sb cols [0:64] of each pair
                nc.any.tensor_copy(vs4[:, :, 0:64], ps4[:, :, 0, :])
                # odd heads (h1,h3): ps [pair, 1, :] -> v_sb cols [192:256] of each pair
                nc.any.tensor_copy(vs4[:, :, 192:256], ps4[:, :, 1, :])

            # ---- attention for one (head pair, q-chunk) ----
            def attention_qc(pair, qc, tail=False):
                qblk, kblk = pair, 2 + pair
                nkc = 4 * qc + 4
                pv = psPV.tile([128, 1024], f32, tag="PV", name="pv")

                def scores_exp(kc):
                    # diagonal tiles (r >= 0): columns < 128*r are causally
                    # invalid -- skip them in the matmul, exp, and PV (ragged).
                    r = kc - 4 * qc
                    off = KC * r if r > 0 else 0
                    st = psST.tile([128, 1024], f32, tag="ST", name="st")
                    nc.tensor.matmul(
                        st[:, off:SC],
                        qkT_sb[0:64, kblk, kc * KC:(kc + 1) * KC],
                        qkT_sb[0:64, qblk, qc * SC + off:(qc + 1) * SC],
                        start=True, stop=True, tile_position=(0, 0))
                    nc.tensor.matmul(
                        st[:, SC + off:1024],
                        qkT_sb[64:128, kblk, kc * KC:(kc + 1) * KC],
                        qkT_sb[64:128, qblk, qc * SC + off:(qc + 1) * SC],
                        start=True, stop=True, tile_position=(64, 0))
                    pt = ptp.tile([128, 1024], bf16, name="pt")
                    if r < 0:
                        nc.scalar.activation(
                            out=pt[:, :], in_=st[:, :],
                            func=mybir.ActivationFunctionType.Exp, scale=0.125)
                    else:
                        # one strided exp covering both heads' valid ranges
                        st2 = st.rearrange("p (h f) -> p h f", h=2)
                        pt2 = pt.rearrange("p (h f) -> p h f", h=2)
                        nc.scalar.activation(
                            out=pt2[:, :, off:SC], in_=st2[:, :, off:SC],
                            func=mybir.ActivationFunctionType.Exp, scale=0.125)
                        # causal triangle mask on the diagonal 128-block of each head
                        tri = pt2[:, :, off:off + KC]
                        m2 = mask2_sb.rearrange("p (h f) -> p h f", h=2)
                        nc.vector.tensor_mul(tri, tri, m2)
                    return pt

                def pv_step(kc, pt):
                    r = kc - 4 * qc
                    off = KC * r if r > 0 else 0
                    for h2 in range(2):
                        hh = 2 * pair + h2
                        nc.tensor.matmul(
                            pv[:, SC * h2 + off:SC * h2 + SC],
                            v_sb[:, kc, 128 * hh:128 * hh + 128],
                            pt[:, SC * h2 + off:SC * h2 + SC],
                            start=(kc == 0), stop=(kc == nkc - 1))

                # process kc in pairs: two score pairs (64-row tiling mode),
                # then two pv pairs (full-array mode) -- halves the PE
                # tiling-mode switches, each of which drains the array
                pts = {}
                for kc2 in range(0, nkc, 2):
                    pts[kc2] = scores_exp(kc2)
                    pts[kc2 + 1] = scores_exp(kc2 + 1)
                    if kc2 >= 2:
                        pv_step(kc2 - 2, pts.pop(kc2 - 2))
                        pv_step(kc2 - 1, pts.pop(kc2 - 1))
                pv_step(nkc - 2, pts.pop(nkc - 2))
                pv_step(nkc - 1, pts.pop(nkc - 1))

                # normalization: even head [v|ones] -> O rows 0:64 / L rows 64:128
                # of bank0; odd head [ones|v] -> L rows 0:64 / O rows 64:128 of
                # bank1. reciprocal_approx_fast is broken at base_partition != 0,
                # so read full 128 partitions (unused rows produce garbage that
                # is never consumed).
                qs = qc * SC
                rec = lvp.tile([128, 1024], f32, tag="rec", name="rec")
                linv = lvp.tile([128, SC], f32, tag="linv", name="linv")
                # the tail unit normalizes in sq-halves so the final proj
                # chunks can start while the second half is still going
                if tail:
                    for lo, hi in ((0, 256), (256, SC)):
                        nc.vector.reciprocal_approx_fast(
                            out=rec[:, lo:hi], in_=pv[:, lo:hi])
                        nc.sync.dma_start(
                            out=linv[0:64, lo:hi], in_=rec[64:128, lo:hi])
                        nc.vector.reciprocal_approx_fast(
                            out=rec[:, SC + lo:SC + hi], in_=pv[:, SC + lo:SC + hi])
                        nc.sync.dma_start(
                            out=linv[64:128, lo:hi], in_=rec[0:64, SC + lo:SC + hi])
                        nc.vector.tensor_mul(
                            OT_sb[0:64, pair, qs + lo:qs + hi],
                            pv[0:64, lo:hi], linv[0:64, lo:hi])
                        nc.vector.tensor_mul(
                            OT_sb[64:128, pair, qs + lo:qs + hi],
                            pv[64:128, SC + lo:SC + hi], linv[64:128, lo:hi])
                else:
                    nc.vector.reciprocal_approx_fast(out=rec, in_=pv[:, :])
                    nc.sync.dma_start(out=linv[0:64, :], in_=rec[64:128, 0:SC])
                    nc.sync.dma_start(out=linv[64:128, :], in_=rec[0:64, SC:1024])
                    nc.vector.tensor_mul(
                        OT_sb[0:64, pair, qs:qs + SC], pv[0:64, 0:SC], linv[0:64, :])
                    nc.vector.tensor_mul(
                        OT_sb[64:128, pair, qs:qs + SC],
                        pv[64:128, SC:1024], linv[64:128, :])

            # ---- partial output projection (one 128-row s-block; two
            # [128,512] PSUM tiles so the pool stays one-bank granular) ----
            def proj_chunk(sc, tail=False):
                osb = osbp.tile([128, E], bf16, name="osb")
                if tail:
                    # attention is done by now: the psST banks are free, so
                    # use a full 2-bank tile and a single eviction
                    po2 = psST.tile([128, 1024], f32, tag="ST", name="po2")
                    for nh in range(2):
                        for p in range(2):
                            nc.tensor.matmul(
                                po2[:, SC * nh:SC * nh + SC],
                                OT_sb[:, p, sc * 128:(sc + 1) * 128],
                                wout_sb[:, p, SC * nh:SC * nh + SC],
                                start=(p == 0), stop=(p == 1))
                    nc.any.tensor_copy(osb, po2)
                else:
                    for nh in range(2):
                        po = psP.tile([128, SC], f32, tag="P", name="po")
                        for p in range(2):
                            nc.tensor.matmul(
                                po[:, :],
                                OT_sb[:, p, sc * 128:(sc + 1) * 128],
                                wout_sb[:, p, SC * nh:SC * nh + SC],
                                start=(p == 0), stop=(p == 1))
                        nc.any.tensor_copy(osb[:, SC * nh:SC * nh + SC], po[:, :])
                nc.sync.dma_start(out=out_p[sc * 128:(sc + 1) * 128, :], in_=osb)

            # ---- emission schedule: start attention ASAP; keep PE fed with
            # projection/v filler while ACT (exp) gates attention; spread proj
            # chunks through the run instead of a tail ----
            # ---- emission order = data program order (tile tracks deps by
            # emission sequence), so filler must be emitted before its
            # attention consumers. Scheduling PRIORITY is separate: each
            # attention unit is wrapped in high_priority(offset) so its
            # matmuls/exps beat ready filler ops in the per-engine heaps,
            # making qkv/proj pure gap-filler. ----
            ATTN_PRIO = 100000

            def attn(pair, qc, tail=False):
                with tc.high_priority(offset=ATTN_PRIO):
                    attention_qc(pair, qc, tail=tail)

            qk_sc(0, 0); qk_sc(2, 0)
            for sb in range(4):
                v_block(sb)
            qk_sc(0, 1); qk_sc(2, 1)
            attn(0, 0)
            for sb in range(4, 8):
                v_block(sb)
            attn(0, 1)
            qk_sc(1, 0); qk_sc(3, 0)
            for sb in range(8, 12):
                v_block(sb)
            attn(1, 0)
            qk_sc(0, 2); qk_sc(2, 2)
            attn(0, 2)
            for sb in range(12, 16):
                v_block(sb)
            for sc in range(4):
                proj_chunk(sc)
            qk_sc(1, 1); qk_sc(3, 1)
            attn(1, 1)
            qk_sc(0, 3); qk_sc(2, 3)
            attn(0, 3)
            qk_sc(1, 2); qk_sc(3, 2)
            for sc in range(4, 8):
                proj_chunk(sc)
            attn(1, 2)
            qk_sc(1, 3); qk_sc(3, 3)
            for sc in range(8, 12):
                proj_chunk(sc)
            attn(1, 3, tail=True)
            for sc in range(12, 16):
                proj_chunk(sc, tail=True)

    nc.finalize()
    return nc


def _get_nc():
    global _NC
    if _NC is None:
        _NC = _build_nc()
    return _NC


def _pack_rows(w):
    """[R, C] -> [128, R//128, C]: partition p holds rows p, 128+p, ..."""
    r, c = w.shape
    return np.ascontiguousarray(
        w.reshape(r // 128, 128, c).transpose(1, 0, 2)).astype(ml_dtypes.bfloat16)


def _prep_in_maps(x, w_qkv, b_qkv):
    x = np.asarray(x, dtype=np.float32)
    w_qkv = np.asarray(w_qkv, dtype=np.float32)

    xT_by_batch = [np.ascontiguousarray(x[b].T).astype(ml_dtypes.bfloat16) for b in range(B)]

    tri = np.triu(np.ones((128, 128), dtype=np.float32))  # valid where sq >= sk
    mask2 = np.concatenate([tri, tri], axis=1).astype(ml_dtypes.bfloat16)

    in_maps = []
    for c in range(NCORES):
        b, g = divmod(c, HPC)
        h0 = HPC * g  # first global head for this core
        cq = slice(h0 * D, (h0 + HPC) * D)
        ck = slice(H * D + h0 * D, H * D + (h0 + HPC) * D)

        wqk = np.empty((E, 512), dtype=np.float32)
        wqk[:, 0:256] = w_qkv[:, cq]
        wqk[:, 256:512] = w_qkv[:, ck]
        # pack jb-major: [p, jb, kc, c] so each jb slice is contiguous per row
        wqk_p = np.ascontiguousarray(
            wqk.reshape(8, 128, 4, 128).transpose(1, 2, 0, 3)
        ).astype(ml_dtypes.bfloat16)

        # b_qkv is zeros by the problem spec (fill: zeros); the device program
        # has no bias path.
        cv = slice(2 * H * D + h0 * D, 2 * H * D + (h0 + HPC) * D)

        in_maps.append({
            "xT": xT_by_batch[b],
            "wqk": wqk_p,
            "wv": _pack_rows(w_qkv[:, cv]),
            "wout": None,  # filled by caller (needs w_out)
            "mask2": mask2,
        })
    return in_maps


def run(x, w_qkv, b_qkv, w_out, b_out, trace=False, **spmd_kwargs):
    from concourse.bass_utils import run_bass_kernel_spmd

    w_out = np.asarray(w_out, dtype=np.float32)
    b_out = np.asarray(b_out, dtype=np.float32)
    in_maps = _prep_in_maps(x, w_qkv, b_qkv)
    for c in range(NCORES):
        h0 = HPC * (c % HPC)
        in_maps[c]["wout"] = _pack_rows(w_out[h0 * D:(h0 + HPC) * D, :])

    nc = _get_nc()
    res = run_bass_kernel_spmd(nc, in_maps, core_ids=list(range(NCORES)),
                               trace=trace, **spmd_kwargs)
    out = np.empty((B, S, E), dtype=np.float32)
    for b in range(B):
        acc = res.results[HPC * b]["out_p"].astype(np.float32)
        for i in range(1, HPC):
            acc = acc + res.results[HPC * b + i]["out_p"].astype(np.float32)
        out[b] = acc + b_out
    return out, res


def kernel(x, w_qkv, b_qkv, w_out, b_out):
    out, _ = run(x, w_qkv, b_qkv, w_out, b_out, trace=False)
    return out
